# revision 1
# baseline (speedup 1.0000x reference)
"""Trainium2 Bass kernel for a pre-norm transformer block (B=16, N=1024, C=768).

Sharding: pure data-parallel over batch across 8 NeuronCores (2 items/core).
All matmuls run as fp32r (full PE rate) except p@v and fc2 which run bf16.

Per-core dataflow (T = 2048 tokens):
  A. LN1 over x (token-major) -> h, PE-transpose -> hT [768, T] (feat-major)
  B. qkv: q,k produced feat-major (lhsT=W, rhs=hT) -> DRAM scratch;
     v produced token-major (lhsT=hT, rhs=W) -> SBUF bf16 with a ones column
     appended per head (v_aug [tok, 12, 65]).
  C. attention per (batch, head-pair): scoresT = kT.T @ qT with K=64
     row-group packing (two heads concurrently in the PE array);
     exp via ACT (scale=1/8 fused) -> bf16 pT [k, q]; o^T = v_aug.T @ pT
     gives 64 rows of head output + row 64 = softmax denominator;
     normalize via K=1 ones-matmul broadcast of 1/denom + DVE multiply.
  D. proj (lhsT=oT) + residual -> x2 -> DRAM scratch.
  E. MLP in 4 token chunks: LN2, transpose, fc1 (fp32r, transposed out),
     exact gelu on ACT -> bf16 gT, fc2 (bf16) + residual -> out.
"""

import numpy as np
from contextlib import ExitStack

import concourse.bass as bass
import concourse.mybir as mybir
import concourse.tile as tile
from concourse import bacc
from concourse.masks import make_identity
from concourse.bass_utils import run_bass_kernel_spmd

DT = mybir.dt
F32, F32R, BF16 = DT.float32, DT.float32r, DT.bfloat16
AF = mybir.ActivationFunctionType
ALU = mybir.AluOpType

B, N, C = 16, 1024, 768
HEADS, HD, HID = 12, 64, 3072
EPS = 1e-5
NCORES = 8
BL = B // NCORES            # 2 batch items per core
T = BL * N                  # 2048 tokens per core
TT = T // 128               # 16 token tiles
KC = C // 128               # 6 contraction chunks over C
HC = HID // 128             # 24 hidden chunks
NPB = N // 128              # 8 token tiles per batch item
PSSC_BUFS, PSOV_BUFS, EPOOL_BUFS = 2, 4, 18
QK_BUFS = 3
PSQK_BUFS, EVQ_BUFS = 4, 4
W1_BUFS, GT_BUFS, X2E_BUFS = KC + 1, HC + 6, 8
R_BUFS, BC_BUFS, OUTP_BUFS = 4, 3, 3
PST2_BUFS, PSF1_BUFS, PSF2_BUFS, GT_BUFS = 2, 4, 2, HC + 6

_CACHE = {}


def _build(flags):
    (ln1_aff, ln2_aff, qkv_bias, proj_bias, fc1_bias, fc2_bias) = flags
    nc = bacc.Bacc("TRN2", target_bir_lowering=False, debug=False,
                   num_devices=NCORES)

    x_d = nc.dram_tensor("x", [BL, N, C], F32, kind="ExternalInput")
    ln1_g_d = nc.dram_tensor("ln1_g", [C], F32, kind="ExternalInput")
    ln1_b_d = nc.dram_tensor("ln1_b", [C], F32, kind="ExternalInput")
    qkv_w_d = nc.dram_tensor("qkv_w", [C, 3 * C], F32R, kind="ExternalInput")
    qkv_b_d = nc.dram_tensor("qkv_b", [3 * C], F32, kind="ExternalInput")
    proj_w_d = nc.dram_tensor("proj_w", [C, C], F32R, kind="ExternalInput")
    proj_b_d = nc.dram_tensor("proj_b", [C], F32, kind="ExternalInput")
    ln2_g_d = nc.dram_tensor("ln2_g", [C], F32, kind="ExternalInput")
    ln2_b_d = nc.dram_tensor("ln2_b", [C], F32, kind="ExternalInput")
    fc1_w_d = nc.dram_tensor("fc1_w", [C, HID], F32R, kind="ExternalInput")
    fc1_b_d = nc.dram_tensor("fc1_b", [HID], F32, kind="ExternalInput")
    fc2_w_d = nc.dram_tensor("fc2_w", [HID, C], F32, kind="ExternalInput")
    fc2_b_d = nc.dram_tensor("fc2_b", [C], F32, kind="ExternalInput")
    out_d = nc.dram_tensor("out", [BL, N, C], F32, kind="ExternalOutput")

    # DRAM scratch
    qkT_d = nc.dram_tensor("qkT_scratch", [2 * KC, 128, T], F32R)  # q rows 0..5, k rows 6..11
    rec_d = nc.dram_tensor("rec_scratch", [BL, KC, 2, 2, 512], F32R)
    x2_d = nc.dram_tensor("x2_scratch", [TT, 128, C], F32)

    x_r = x_d.ap().flatten_outer_dims().rearrange("(t p) c -> t p c", p=128)
    out_r = out_d.ap().flatten_outer_dims().rearrange("(t p) c -> t p c", p=128)

    def bcast_row(dram_ap):
        # DRAM row [L] replicated across 128 partitions
        return bass.AP(tensor=dram_ap.tensor, offset=dram_ap.offset,
                       ap=[[0, 128]] + [list(p) for p in dram_ap.ap])

    with tile.TileContext(nc) as tc, ExitStack() as top:
        const = top.enter_context(tc.tile_pool(name="const", bufs=1))
        identity_f = const.tile([128, 128], F32)
        make_identity(nc, identity_f)
        identity = const.tile([128, 128], F32R)
        nc.vector.tensor_copy(identity, identity_f)
        ones_row_f = const.tile([128, 64], F32)
        nc.vector.memset(ones_row_f, 1.0)
        ones_row = const.tile([128, 64], F32R)
        nc.vector.tensor_copy(ones_row, ones_row_f)

        # broadcast bias/scale tiles (only when the generic paths are on)
        if ln1_aff:
            ln1g_bc = const.tile([128, C], F32)
            nc.sync.dma_start(out=ln1g_bc, in_=bcast_row(ln1_g_d[:]))
            ln1b_bc = const.tile([128, C], F32)
            nc.sync.dma_start(out=ln1b_bc, in_=bcast_row(ln1_b_d[:]))
        if ln2_aff:
            ln2g_bc = const.tile([128, C], F32)
            nc.sync.dma_start(out=ln2g_bc, in_=bcast_row(ln2_g_d[:]))
            ln2b_bc = const.tile([128, C], F32)
            nc.sync.dma_start(out=ln2b_bc, in_=bcast_row(ln2_b_d[:]))
        if qkv_bias:
            qkvb_pp = const.tile([128, 2 * KC], F32)  # per-partition for q,k rows
            nc.sync.dma_start(out=qkvb_pp,
                              in_=qkv_b_d[0:2 * C].rearrange("(c p) -> p c", p=128))
            vb_bc = const.tile([128, C], F32)
            nc.sync.dma_start(out=vb_bc, in_=bcast_row(qkv_b_d[2 * C:3 * C]))
        if proj_bias:
            projb_bc = const.tile([128, C], F32)
            nc.sync.dma_start(out=projb_bc, in_=bcast_row(proj_b_d[:]))
        if fc1_bias:
            fc1b_pp = const.tile([128, HC], F32)
            nc.sync.dma_start(out=fc1b_pp,
                              in_=fc1_b_d[:].rearrange("(c p) -> p c", p=128))
        if fc2_bias:
            fc2b_bc = const.tile([128, C], F32)
            nc.sync.dma_start(out=fc2b_bc, in_=bcast_row(fc2_b_d[:]))

        # persistent pools, LIFO-ordered: vaug spans phases A-D (bottom),
        # hT spans A-B, oT spans C-D
        mid = ExitStack()
        vaug_pool = mid.enter_context(tc.tile_pool(name="vaug", bufs=1))
        v_aug = [vaug_pool.tile([128, HEADS, HD + 1], BF16, name=f"vaug{t}")
                 for t in range(TT)]
        for t in range(TT):
            nc.vector.memset(v_aug[t][:, :, HD:HD + 1], 1.0)
        pwpool = mid.enter_context(tc.tile_pool(name="pw", bufs=KC))
        pw = [pwpool.tile([128, C], F32R, tag="pw", name=f"pw{s}")
              for s in range(KC)]
        hT_stack = ExitStack()
        hT_pool = hT_stack.enter_context(tc.tile_pool(name="hT", bufs=1))
        hT = [hT_pool.tile([128, T], F32R, name=f"hT{f}") for f in range(KC)]
        wpool = hT_stack.enter_context(tc.tile_pool(name="qkvw", bufs=KC))
        w_s = [wpool.tile([128, 3 * C], F32R, tag="w", name=f"qkvw{s}")
               for s in range(KC)]

        def layer_norm_tile(pools, x_t, affine, g_bc, b_bc, out_dtype):
            """x_t [128, C] fp32 -> normalized tile [128, C] out_dtype."""
            stats, scratch, hpool = pools
            ssum = stats.tile([128, 1], F32, tag="ssum")
            nc.vector.tensor_reduce(ssum, x_t, axis=mybir.AxisListType.X,
                                    op=ALU.add)
            sq = scratch.tile([128, C], F32, tag="sq")
            sumsq = stats.tile([128, 1], F32, tag="sumsq")
            nc.scalar.activation(sq, x_t, AF.Square, accum_out=sumsq)
            negmu = stats.tile([128, 1], F32, tag="negmu")
            nc.vector.tensor_scalar_mul(negmu, ssum, -1.0 / C)
            musq = stats.tile([128, 1], F32, tag="musq")
            nc.vector.tensor_tensor(musq, negmu, negmu, op=ALU.mult)
            var = stats.tile([128, 1], F32, tag="var")
            # var = sumsq/C - mu^2 + eps  (eps folded via second scalar add)
            nc.vector.tensor_scalar(var, sumsq, 1.0 / C, None, op0=ALU.mult)
            nc.vector.tensor_tensor(var, var, musq, op=ALU.subtract)
            nc.vector.tensor_scalar_add(var, var, EPS)
            std = stats.tile([128, 1], F32, tag="std")
            nc.scalar.sqrt(std, var)
            rstd = stats.tile([128, 1], F32, tag="rstd")
            nc.vector.reciprocal(rstd, std)
            h_t = hpool.tile([128, C], out_dtype, tag="h")
            nc.vector.tensor_scalar(h_t, x_t, negmu, rstd,
                                    op0=ALU.add, op1=ALU.mult)
            if affine:
                nc.vector.tensor_tensor(h_t, h_t, g_bc, op=ALU.mult)
                nc.vector.tensor_tensor(h_t, h_t, b_bc, op=ALU.add)
            return h_t

        # ---------------- Phase A: LN1 + transpose -> hT ----------------
        with nc.named_scope("ln1"), ExitStack() as ph:
            xpool = ph.enter_context(tc.tile_pool(name="xa", bufs=3))
            scratch = ph.enter_context(tc.tile_pool(name="scr", bufs=2))
            stats = ph.enter_context(tc.tile_pool(name="stats", bufs=4))
            hpool = ph.enter_context(tc.tile_pool(name="h", bufs=3))
            pst = ph.enter_context(tc.tile_pool(name="pst", bufs=4, space="PSUM"))
            for t in range(TT):
                x_t = xpool.tile([128, C], F32, tag="x")
                nc.sync.dma_start(out=x_t, in_=x_r[t])
                h_t = layer_norm_tile(
                    (stats, scratch, hpool), x_t, ln1_aff,
                    ln1g_bc if ln1_aff else None,
                    ln1b_bc if ln1_aff else None, F32R)
                for f in range(KC):
                    ps_t = pst.tile([128, 128], F32R, tag="pt")
                    nc.tensor.transpose(ps_t, h_t[:, f * 128:(f + 1) * 128],
                                        identity)
                    dst = hT[f][:, t * 128:(t + 1) * 128]
                    if f % 2 == 0:
                        nc.vector.tensor_copy(dst, ps_t)
                    else:
                        nc.scalar.copy(dst, ps_t)

        for s in range(KC):
            nc.sync.dma_start(out=w_s[s], in_=qkv_w_d[s * 128:(s + 1) * 128, :])

        # ---------------- Phase B: qkv ----------------
        with nc.named_scope("qkv"), ExitStack() as ph:
            psqk = ph.enter_context(tc.tile_pool(name="psqk", bufs=PSQK_BUFS, space="PSUM"))
            evq = ph.enter_context(tc.tile_pool(name="evq", bufs=EVQ_BUFS))
            # q,k feat-major -> DRAM scratch
            for fc in range(2 * KC):
                for tc4 in range(T // 512):
                    ps = psqk.tile([128, 512], F32, tag="ps")
                    for s in range(KC):
                        nc.tensor.matmul(ps, w_s[s][:, fc * 128:(fc + 1) * 128],
                                         hT[s][:, tc4 * 512:(tc4 + 1) * 512],
                                         start=(s == 0), stop=(s == KC - 1))
                    qk_sb = evq.tile([128, 512], F32R, tag="qk")
                    if qkv_bias:
                        nc.scalar.activation(qk_sb, ps, AF.Identity,
                                             bias=qkvb_pp[:, fc:fc + 1])
                    else:
                        nc.vector.tensor_copy(qk_sb, ps)
                    nc.sync.dma_start(
                        out=qkT_d[fc, :, tc4 * 512:(tc4 + 1) * 512], in_=qk_sb)
            # v token-major -> v_aug bf16
            for t in range(TT):
                for off, sz, h_lo, h_hi in ((2 * C, 384, 0, 6),
                                            (2 * C + 384, 384, 6, 12)):
                    ps = psqk.tile([128, 512], F32, tag="ps")
                    for s in range(KC):
                        nc.tensor.matmul(ps[:, 0:sz],
                                         hT[s][:, t * 128:(t + 1) * 128],
                                         w_s[s][:, off:off + sz],
                                         start=(s == 0), stop=(s == KC - 1))
                    dst = v_aug[t][:, h_lo:h_hi, 0:HD]
                    if qkv_bias:
                        nc.vector.scalar_tensor_tensor(
                            dst, ps[:, 0:sz], 1.0,
                            vb_bc[:, off - 2 * C:off - 2 * C + sz],
                            op0=ALU.mult, op1=ALU.add)
                    else:
                        nc.vector.tensor_copy(dst, ps[:, 0:sz])

        for s in range(KC):
            nc.sync.dma_start(out=pw[s], in_=proj_w_d[s * 128:(s + 1) * 128, :])

        # ---------------- Phase C: attention ----------------
        hT_stack.close()
        oT_pool = mid.enter_context(tc.tile_pool(name="oT", bufs=1))
        oT = [oT_pool.tile([128, T], F32R, name=f"oT{f}") for f in range(KC)]
        with nc.named_scope("attn"), ExitStack() as ph:
            qkpool = ph.enter_context(tc.tile_pool(name="qk", bufs=QK_BUFS))
            epool = ph.enter_context(tc.tile_pool(name="exp", bufs=EPOOL_BUFS))
            pssc = ph.enter_context(tc.tile_pool(name="pssc", bufs=PSSC_BUFS, space="PSUM"))
            psov = ph.enter_context(tc.tile_pool(name="psov", bufs=PSOV_BUFS, space="PSUM"))
            rpool = ph.enter_context(tc.tile_pool(name="rec", bufs=R_BUFS))
            bcpool = ph.enter_context(tc.tile_pool(name="bcp", bufs=BC_BUFS))
            xres = ph.enter_context(tc.tile_pool(name="xres", bufs=3))
            x2pool = ph.enter_context(tc.tile_pool(name="x2", bufs=3))
            for b in range(BL):
                for hp in range(HEADS // 2):
                    kT_p = qkpool.tile([128, N], F32R, tag="kT")
                    nc.sync.dma_start(out=kT_p,
                                      in_=qkT_d[KC + hp, :, b * N:(b + 1) * N])
                    qT_p = qkpool.tile([128, N], F32R, tag="qT")
                    nc.sync.dma_start(out=qT_p,
                                      in_=qkT_d[hp, :, b * N:(b + 1) * N])
                    eAB = []
                    for kt in range(NPB):
                        # two psum tiles per kt (head A, head B); the two
                        # score matmuls target distinct PE row groups and run
                        # concurrently in silicon
                        psS = pssc.tile([128, N], F32, tag="s")
                        psB = pssc.tile([128, N], F32, tag="s")
                        for qc in range(N // 512):
                            qs = slice(qc * 512, (qc + 1) * 512)
                            nc.tensor.matmul(
                                psS[:, qs], kT_p[0:64, kt * 128:(kt + 1) * 128],
                                qT_p[0:64, qs], start=True, stop=True,
                                tile_position=(0, 0))
                            nc.tensor.matmul(
                                psB[:, qs], kT_p[64:128, kt * 128:(kt + 1) * 128],
                                qT_p[64:128, qs], start=True, stop=True,
                                tile_position=(64, 0))
                        eA_t = epool.tile([128, N], BF16, tag="e")
                        nc.scalar.activation(eA_t, psS, AF.Exp, scale=HD ** -0.5)
                        eB_t = epool.tile([128, N], BF16, tag="e")
                        nc.scalar.activation(eB_t, psB, AF.Exp, scale=HD ** -0.5)
                        eAB.append((eA_t, eB_t))
                    for qc in range(N // 512):
                        qs = slice(qc * 512, (qc + 1) * 512)
                        psoA = psov.tile([128, 512], F32, tag="o")
                        psoB = psov.tile([128, 512], F32, tag="o")
                        for ps_o, ih, h in ((psoA, 0, 2 * hp),
                                            (psoB, 1, 2 * hp + 1)):
                            es = slice(qc * 512, (qc + 1) * 512)
                            for kt in range(NPB):
                                nc.tensor.matmul(
                                    ps_o[0:HD + 1, :],
                                    v_aug[b * NPB + kt][:, h, :],
                                    eAB[kt][ih][:, es],
                                    start=(kt == 0), stop=(kt == NPB - 1))
                        recA = rpool.tile([1, 512], F32R, tag="rA")
                        recB = rpool.tile([1, 512], F32R, tag="rB")
                        with nc.allow_low_precision(reason="fp32r is fp32 bits"):
                            nc.vector.reciprocal(recA, psoA[HD:HD + 1, :])
                            nc.vector.reciprocal(recB, psoB[HD:HD + 1, :])
                        nc.sync.dma_start(out=rec_d[b, hp, qc, 0], in_=recA)
                        nc.sync.dma_start(out=rec_d[b, hp, qc, 1], in_=recB)
                        bc_sb = bcpool.tile([128, 512], F32R, tag="bc")
                        rAd = rec_d[b, hp, qc, 0]
                        rBd = rec_d[b, hp, qc, 1]
                        nc.sync.dma_start(
                            out=bc_sb[0:64, :],
                            in_=bass.AP(tensor=rAd.tensor, offset=rAd.offset,
                                        ap=[[0, 64]] + [list(p) for p in rAd.ap]))
                        nc.sync.dma_start(
                            out=bc_sb[64:128, :],
                            in_=bass.AP(tensor=rBd.tensor, offset=rBd.offset,
                                        ap=[[0, 64]] + [list(p) for p in rBd.ap]))
                        gqs = slice(b * N + qc * 512, b * N + (qc + 1) * 512)
                        nc.vector.tensor_tensor(
                            oT[hp][0:64, gqs], psoA[0:HD, :],
                            bc_sb[0:64, :], op=ALU.mult)
                        nc.vector.tensor_tensor(
                            oT[hp][64:128, gqs], psoB[0:HD, :],
                            bc_sb[64:128, :], op=ALU.mult)
                # proj + residual for this batch item (overlaps the other
                # batch item's attention on the other engines)
                for t in range(b * NPB, (b + 1) * NPB):
                    x_t = xres.tile([128, C], F32, tag="xr")
                    nc.sync.dma_start(out=x_t, in_=x_r[t])
                    x2_sb = x2pool.tile([128, C], F32, tag="x2")
                    for off, sz in ((0, 384), (384, 384)):
                        ps = psov.tile([128, 512], F32, tag="o")
                        for s in range(KC):
                            nc.tensor.matmul(ps[:, 0:sz],
                                             oT[s][:, t * 128:(t + 1) * 128],
                                             pw[s][:, off:off + sz],
                                             start=(s == 0), stop=(s == KC - 1))
                        nc.vector.scalar_tensor_tensor(
                            x2_sb[:, off:off + sz], ps[:, 0:sz], 1.0,
                            x_t[:, off:off + sz], op0=ALU.mult, op1=ALU.add)
                    if proj_bias:
                        nc.vector.tensor_tensor(x2_sb, x2_sb, projb_bc, op=ALU.add)
                    nc.sync.dma_start(out=x2_d[t], in_=x2_sb)

        # ---------------- Phase E: MLP ----------------
        mid.close()
        with nc.named_scope("mlp"), ExitStack() as ph:
            w2pool = ph.enter_context(tc.tile_pool(name="w2", bufs=HC + 2))
            w2 = []
            for s in range(HC):
                w_t = w2pool.tile([128, C], BF16, tag="w2", name=f"w2_{s}")
                nc.gpsimd.dma_start(out=w_t, in_=fc2_w_d[s * 128:(s + 1) * 128, :])
                w2.append(w_t)
            x2pool = ph.enter_context(tc.tile_pool(name="x2e", bufs=X2E_BUFS))
            scratch = ph.enter_context(tc.tile_pool(name="scr2", bufs=2))
            stats = ph.enter_context(tc.tile_pool(name="stats2", bufs=4))
            h2pool = ph.enter_context(tc.tile_pool(name="h2", bufs=3))
            h2Tpool = ph.enter_context(tc.tile_pool(name="h2T", bufs=2 * KC))
            w1pool = ph.enter_context(tc.tile_pool(name="w1", bufs=W1_BUFS))
            gpool = ph.enter_context(tc.tile_pool(name="gT", bufs=GT_BUFS))
            outpool = ph.enter_context(tc.tile_pool(name="outp", bufs=OUTP_BUFS))
            pst = ph.enter_context(tc.tile_pool(name="pst2", bufs=PST2_BUFS, space="PSUM"))
            psf1 = ph.enter_context(tc.tile_pool(name="psf1", bufs=PSF1_BUFS, space="PSUM"))
            psf2 = ph.enter_context(tc.tile_pool(name="psf2", bufs=PSF2_BUFS, space="PSUM"))
            CH = 4                      # token chunks
            CT = T // CH // 128         # 4 token tiles per chunk
            for ch in range(CH):
                x2_t = []
                h2T_c = [h2Tpool.tile([128, CT * 128], F32R, tag="h2T",
                                      name=f"h2T{ch}_{f}") for f in range(KC)]
                for lt in range(CT):
                    t = ch * CT + lt
                    x_t = x2pool.tile([128, C], F32, tag="x2e")
                    nc.sync.dma_start(out=x_t, in_=x2_d[t])
                    x2_t.append(x_t)
                    h2 = layer_norm_tile(
                        (stats, scratch, h2pool), x_t, ln2_aff,
                        ln2g_bc if ln2_aff else None,
                        ln2b_bc if ln2_aff else None, F32R)
                    for f in range(KC):
                        ps_t = pst.tile([128, 128], F32R, tag="pt2")
                        nc.tensor.transpose(ps_t, h2[:, f * 128:(f + 1) * 128],
                                            identity)
                        nc.vector.tensor_copy(
                            h2T_c[f][:, lt * 128:(lt + 1) * 128], ps_t)
                gT = []
                for half in range(2):
                    w1_s = []
                    for s in range(KC):
                        w_t = w1pool.tile([128, HID // 2], F32R, tag="w1")
                        nc.sync.dma_start(
                            out=w_t,
                            in_=fc1_w_d[s * 128:(s + 1) * 128,
                                        half * (HID // 2):(half + 1) * (HID // 2)])
                        w1_s.append(w_t)
                    for hc in range(HC // 2):
                        ps = psf1.tile([128, 512], F32, tag="f1")
                        for s in range(KC):
                            nc.tensor.matmul(
                                ps, w1_s[s][:, hc * 128:(hc + 1) * 128],
                                h2T_c[s][:, :], start=(s == 0),
                                stop=(s == KC - 1))
                        g_t = gpool.tile([128, 512], BF16, tag="g")
                        if fc1_bias:
                            nc.scalar.activation(
                                g_t, ps, AF.Gelu,
                                bias=fc1b_pp[:, half * 12 + hc:half * 12 + hc + 1])
                        else:
                            nc.scalar.activation(g_t, ps, AF.Gelu)
                        gT.append(g_t)
                for lt in range(CT):
                    t = ch * CT + lt
                    out_sb = outpool.tile([128, C], F32, tag="out")
                    for off, sz in ((0, 384), (384, 384)):
                        ps = psf2.tile([128, 384], F32, tag="f2")
                        for s in range(HC):
                            nc.tensor.matmul(ps[:, 0:sz],
                                             gT[s][:, lt * 128:(lt + 1) * 128],
                                             w2[s][:, off:off + sz],
                                             start=(s == 0), stop=(s == HC - 1))
                        nc.vector.scalar_tensor_tensor(
                            out_sb[:, off:off + sz], ps[:, 0:sz], 1.0,
                            x2_t[lt][:, off:off + sz], op0=ALU.mult, op1=ALU.add)
                    if fc2_bias:
                        nc.vector.tensor_tensor(out_sb, out_sb, fc2b_bc,
                                                op=ALU.add)
                    nc.sync.dma_start(out=out_r[ch * CT + lt], in_=out_sb)

    nc.finalize()
    return nc


def _get_nc(flags):
    if flags not in _CACHE:
        _CACHE[flags] = _build(flags)
    return _CACHE[flags]


def kernel(**inputs):
    inp = {k: np.ascontiguousarray(np.asarray(v, dtype=np.float32))
           for k, v in inputs.items()}
    flags = (
        not (np.all(inp["ln1_g"] == 1.0) and np.all(inp["ln1_b"] == 0.0)),
        not (np.all(inp["ln2_g"] == 1.0) and np.all(inp["ln2_b"] == 0.0)),
        bool(np.any(inp["qkv_b"] != 0.0)),
        bool(np.any(inp["proj_b"] != 0.0)),
        bool(np.any(inp["fc1_b"] != 0.0)),
        bool(np.any(inp["fc2_b"] != 0.0)),
    )
    nc = _get_nc(flags)
    x = inp["x"]
    shared = {k: v for k, v in inp.items() if k != "x"}
    in_maps = [dict(shared, x=x[i * BL:(i + 1) * BL]) for i in range(NCORES)]
    res = run_bass_kernel_spmd(nc, in_maps, core_ids=list(range(NCORES)))
    out = np.concatenate([res.results[i]["out"] for i in range(NCORES)], axis=0)
    return out.astype(np.float32)

